# revision 15
# baseline (speedup 1.0000x reference)
"""Trainium2 Bass kernel for a pre-LN transformer encoder layer (v2).

Contract: kernel(**inputs) takes the FULL inputs (x [1,4096,1024] plus
weights/biases) and returns the FULL output [1,4096,1024].

Sharding: sequence-parallel over 8 NeuronCores (512 rows each). v2 runs
every large GEMM except the attention scores in fp8e4m3 DoubleRow mode
(2x MACs/instruction via 256-deep contraction), with weights pre-scaled
x32 and pre-interleaved into contraction pairs on the host. K and V are
gathered in fp8 via three sub-AllGathers issued as soon as their data is
staged, hiding the wire time under the remaining projections and early
attention. The softmax exp runs on the ACT engine (the attention-phase
bottleneck) and writes fp8 directly; softmax row-sums come from a fused
ones-column in V. Residuals stay f32.
"""

import numpy as np
import ml_dtypes
from contextlib import ExitStack

import concourse.bass as bass
import concourse.mybir as mybir
import concourse.tile as tile
from concourse import bacc
from concourse.bass_utils import run_bass_kernel_spmd
from concourse.masks import make_identity

P = 128
NCORES = 8
S = 4096
SL = S // NCORES          # 512 local rows
D = 1024
H = 16
DK = D // H               # 64
F = 4096
EPS = 1e-6
WS = 32.0                 # fp8 weight pre-scale
EV = 65                   # per-head V width (64 dims + ones col)
E65 = H * EV              # V row width

F32 = mybir.dt.float32
F32R = mybir.dt.float32r
BF16 = mybir.dt.bfloat16
FP8 = mybir.dt.float8e4
NP_FP8 = ml_dtypes.float8_e4m3
AF = mybir.ActivationFunctionType
OP = mybir.AluOpType
DR = mybir.MatmulPerfMode.DoubleRow

_CACHE = {}


def _build(ln1_a, ln1_b, ln2_a, ln2_b):
    nc = bacc.Bacc("TRN2", target_bir_lowering=False, debug=False,
                   num_devices=NCORES)

    x_d = nc.dram_tensor("x_loc", [SL, D], F32, kind="ExternalInput")
    # weights: [4 pair-groups, 128, 2, out] fp8, pre-scaled x32
    wq_d = nc.dram_tensor("Wq8", [4, P, 2, D], FP8, kind="ExternalInput")
    wk_d = nc.dram_tensor("Wk8", [4, P, 2, D], FP8, kind="ExternalInput")
    wv_d = nc.dram_tensor("Wv8", [4, P, 2, D], FP8, kind="ExternalInput")
    wo_d = nc.dram_tensor("Wo16", [8, P, D], BF16, kind="ExternalInput")
    w1_d = nc.dram_tensor("W18", [4, P, 2, F], FP8, kind="ExternalInput")
    w2_d = nc.dram_tensor("W216", [32, P, D], BF16, kind="ExternalInput")
    # biases pre-scaled x32 (x1024 for b2), see kernel()
    bq_d = nc.dram_tensor("bq32", [D], F32, kind="ExternalInput")
    bk_d = nc.dram_tensor("bk32", [D], F32, kind="ExternalInput")
    bv_d = nc.dram_tensor("bv32", [D], F32, kind="ExternalInput")
    bo_d = nc.dram_tensor("bo32", [D], F32, kind="ExternalInput")
    b1_d = nc.dram_tensor("b1_32", [F], F32, kind="ExternalInput")
    b2_d = nc.dram_tensor("b2_1024", [D], F32, kind="ExternalInput")
    y_d = nc.dram_tensor("y_loc", [SL, D], F32, kind="ExternalOutput")

    with tile.TileContext(nc) as tc, ExitStack() as ctx:
        const = ctx.enter_context(tc.tile_pool(name="const", bufs=1))
        stat = ctx.enter_context(tc.tile_pool(name="stat", bufs=4))
        tmp = ctx.enter_context(tc.tile_pool(name="tmp", bufs=2))
        dram = ctx.enter_context(tc.tile_pool(name="dram", bufs=1, space="DRAM"))

        # CC warm-up: a tiny gather issued first pulls the one-time
        # collective-init barrier (~110us) to the front of the kernel.
        dumb = dram.tile([64], FP8)
        gdumb = dram.tile([NCORES, 64], FP8, addr_space="Shared")
        nc.gpsimd.collective_compute(
            "AllGather", mybir.AluOpType.bypass,
            replica_groups=[list(range(NCORES))],
            ins=[dumb.opt()], outs=[gdumb.opt()])

        # ---------------- constants ----------------
        ident = const.tile([P, P], F32)
        make_identity(nc, ident)
        ones16 = const.tile([P, 16], F32)
        nc.vector.memset(ones16[:], 1.0)
        # E65[k, m]: row 0 selects m<64 (head A), row 64 selects m>=64 (head B)
        e65_f = const.tile([65, P], F32)
        nc.vector.memset(e65_f[:], 0.0)
        nc.vector.memset(e65_f[0:1, 0:64], 1.0)
        nc.vector.memset(e65_f[64:65, 64:128], 1.0)
        e65 = const.tile([65, P], F32R)
        nc.vector.tensor_copy(e65[:], e65_f[:])
        rc65_f = const.tile([65, SL], F32)
        nc.vector.memset(rc65_f[:], 1.0)
        ones_f = const.tile([65, P], F32)
        nc.vector.memset(ones_f[:], 1.0)
        ones65 = const.tile([65, P], F32R)
        nc.vector.tensor_copy(ones65[:], ones_f[:])

        bq_t = const.tile([P, 8], F32)
        nc.sync.dma_start(bq_t[:], bq_d.rearrange("(c p) -> p c", p=P))
        bk_t = const.tile([P, 8], F32)
        nc.sync.dma_start(bk_t[:], bk_d.rearrange("(c p) -> p c", p=P))
        b1_t = const.tile([P, 32], F32)
        nc.sync.dma_start(b1_t[:], b1_d.rearrange("(c p) -> p c", p=P))

        rcon_f = const.tile([65, D], F32)
        nc.sync.dma_start(rcon_f[0:1, :], bv_d[None, :])
        nc.sync.dma_start(rcon_f[32:33, :], bo_d[None, :])
        nc.sync.dma_start(rcon_f[64:65, :], b2_d[None, :])
        rcon = const.tile([65, D], F32R)
        nc.vector.tensor_copy(rcon[:], rcon_f[:])
        bvr = rcon[0:1, :]
        bor = rcon[32:33, :]
        b2r = rcon[64:65, :]

        def ln_j(src_big, j, a_val, b_val, hT8, tp_psum):
                xt = src_big[:, j, :]
                mu = stat.tile([P, 1], F32, name=f"mu{j}", tag="mu")
                nc.vector.reduce_sum(mu[:], xt, axis=mybir.AxisListType.X)
                nc.vector.tensor_scalar_mul(mu[:], mu[:], 1.0 / D)
                xc = tmp.tile([P, D], F32, name=f"xc{j}", tag="xc")
                nc.vector.tensor_scalar(xc[:], xt, mu[:], None, OP.subtract)
                sq = tmp.tile([P, D], F32, name=f"sq{j}", tag="h")
                nc.vector.tensor_tensor(sq[:], xc[:], xc[:], OP.mult)
                var = stat.tile([P, 1], F32, name=f"var{j}", tag="var")
                nc.vector.reduce_sum(var[:], sq[:], axis=mybir.AxisListType.X)
                std = stat.tile([P, 1], F32, name=f"std{j}", tag="std")
                nc.scalar.activation(std[:], var[:], AF.Sqrt, scale=1.0 / (D - 1))
                nc.vector.tensor_scalar_add(std[:], std[:], EPS)
                r = stat.tile([P, 1], F32, name=f"r{j}", tag="r")
                nc.vector.reciprocal(r[:], std[:])
                nc.vector.tensor_scalar_mul(r[:], r[:], float(a_val))
                h = tmp.tile([P, D], F32, name=f"h{j}", tag="h")
                nc.vector.tensor_scalar(h[:], xc[:], r[:], float(b_val),
                                        OP.mult, OP.add)
                for c0 in (0, 4):
                    tp = tp_psum.tile([P, 512], F32, name=f"tp{j}_{c0}",
                                      tag="tp")
                    for ci in range(4):
                        cc = c0 + ci
                        nc.tensor.transpose(tp[:, ci * P:(ci + 1) * P],
                                            h[:, cc * P:(cc + 1) * P], ident[:])
                    nc.vector.tensor_copy(
                        hT8[:, c0:c0 + 4, j * P:(j + 1) * P],
                        tp.rearrange("p (a b) -> p a b", a=4))

        def layer_norm_to_T(src_big, a_val, b_val, hT8, tp_psum):
            """src_big [P, 4, D] f32 -> hT8 [P, 8, SL] fp8 (transposed LN)."""
            for j in range(4):
                ln_j(src_big, j, a_val, b_val, hT8, tp_psum)

        # bounce + gather buffers (fp8)
        KB1 = dram.tile([512, SL], FP8)
        KB2 = dram.tile([512, SL], FP8)
        VB1 = dram.tile([SL, E65 // 2], BF16)
        VB2 = dram.tile([SL, E65 // 2], BF16)
        GK1 = dram.tile([NCORES, 512, SL], FP8, addr_space="Shared")
        GK2 = dram.tile([NCORES, 512, SL], FP8, addr_space="Shared")
        GV1 = dram.tile([NCORES, SL, E65 // 2], BF16, addr_space="Shared")
        GV2 = dram.tile([NCORES, SL, E65 // 2], BF16, addr_space="Shared")
        groups = [list(range(NCORES))]

        # W1 stream pool: DMAs prefetch during attention
        w1pool = ctx.enter_context(tc.tile_pool(name="w1pool", bufs=16))
        x2_pool = ctx.enter_context(tc.tile_pool(name="x2_pool", bufs=1))
        h2T_pool = ctx.enter_context(tc.tile_pool(name="h2T_pool", bufs=1))
        h2T8 = h2T_pool.tile([P, 8, SL], FP8)

        with (
            tc.tile_pool(name="x_pool", bufs=1) as x_pool,
            tc.tile_pool(name="ctx_pool", bufs=1) as ctx_pool,
        ):
            x_big = x_pool.tile([P, 4, D], F32)
            with tc.tile_pool(name="qt_pool", bufs=1) as qt_pool:
                QT8 = qt_pool.tile([P, 8, SL], FP8)

                # ---------------- phase 1: LN1 + transpose ----------------
                with tc.tile_pool(name="hT_pool", bufs=1) as hT_pool:
                    hT8 = hT_pool.tile([P, 8, SL], FP8)
                    with tc.tile_pool(name="tp1", bufs=2, space="PSUM") as tpp:
                        nc.sync.dma_start(
                            x_big[:],
                            x_d.rearrange("(j p) d -> p j d", p=P))
                        layer_norm_to_T(x_big, ln1_a, ln1_b, hT8, tpp)

                    # ------- phase 2: K/V/Q projections + sub-gathers ------
                    with (
                        tc.tile_pool(name="wbig", bufs=9) as wbig,
                        tc.tile_pool(name="kvstage", bufs=2) as kvstage,
                        tc.tile_pool(name="qkps", bufs=2, space="PSUM") as qkps,
                    ):
                        wkt = []
                        for a in range(4):
                            w = wbig.tile([P, 2, D], FP8, name=f"wk{a}",
                                          tag="wbig")
                            nc.sync.dma_start(w[:], wk_d[a])
                            wkt.append(w)

                        def k_proj(dc):
                            ps = qkps.tile([P, SL], F32, name=f"kps{dc}",
                                           tag="qk")
                            for a in range(4):
                                nc.tensor.matmul(
                                    ps[:], wkt[a][:, :, dc * P:(dc + 1) * P],
                                    hT8[:, 2 * a:2 * a + 2, :],
                                    start=(a == 0), stop=(a == 3),
                                    perf_mode=DR)
                            kstg = kvstage.tile([P, SL], FP8,
                                                name=f"kstg{dc}", tag="kstg")
                            nc.vector.tensor_scalar(kstg[:], ps[:],
                                                    bk_t[:, dc:dc + 1],
                                                    1.0 / WS, OP.add, OP.mult)
                            KB = KB1 if dc < 4 else KB2
                            nc.sync.dma_start(
                                KB[(dc % 4) * P:(dc % 4 + 1) * P, :], kstg[:])

                        for dc in range(4):
                            k_proj(dc)
                        nc.gpsimd.collective_compute(
                            "AllGather", OP.bypass, replica_groups=groups,
                            ins=[KB1.opt()], outs=[GK1.opt()])

                        wvt = []
                        for a in range(4):
                            w = wbig.tile([P, 2, D], FP8, name=f"wv{a}",
                                          tag="wbig")
                            nc.sync.dma_start(w[:], wv_d[a])
                            wvt.append(w)
                        for sb in range(4):
                            vstg = kvstage.tile([P, E65], BF16,
                                                name=f"vstg{sb}", tag="vstg")
                            vview = vstg.rearrange("p (h e) -> p h e", e=EV)
                            for nb in range(2):
                                ps = qkps.tile([P, 512], F32,
                                               name=f"vps{sb}_{nb}", tag="qk")
                                for a in range(4):
                                    nc.tensor.matmul(
                                        ps[:],
                                        hT8[:, 2 * a:2 * a + 2,
                                            sb * P:(sb + 1) * P],
                                        wvt[a][:, :, nb * 512:(nb + 1) * 512],
                                        start=(a == 0), stop=False,
                                        perf_mode=DR)
                                nc.tensor.matmul(
                                    ps[:], ones65[0:1, :],
                                    bvr[:, nb * 512:(nb + 1) * 512],
                                    start=False, stop=True)
                                nc.vector.tensor_scalar(
                                    vview[:, nb * 8:(nb + 1) * 8, 0:64],
                                    ps.rearrange("p (h d) -> p h d", d=64),
                                    1.0 / WS, None, OP.mult)
                            nc.vector.tensor_copy(vview[:, :, 64], ones16[:])
                            nc.sync.dma_start(
                                VB1[sb * P:(sb + 1) * P, :],
                                vstg[:, 0:E65 // 2])
                            nc.sync.dma_start(
                                VB2[sb * P:(sb + 1) * P, :],
                                vstg[:, E65 // 2:E65])
                        nc.gpsimd.collective_compute(
                            "AllGather", OP.bypass, replica_groups=groups,
                            ins=[VB1.opt()], outs=[GV1.opt()])
                        nc.gpsimd.collective_compute(
                            "AllGather", OP.bypass, replica_groups=groups,
                            ins=[VB2.opt()], outs=[GV2.opt()])

                        for dc in range(4, 8):
                            k_proj(dc)
                        nc.gpsimd.collective_compute(
                            "AllGather", OP.bypass, replica_groups=groups,
                            ins=[KB2.opt()], outs=[GK2.opt()])

                        # Q last: overlaps the gathers. QT8 stores Q/2.
                        wqt = []
                        for a in range(4):
                            w = wbig.tile([P, 2, D], FP8, name=f"wq{a}",
                                          tag="wbig")
                            nc.sync.dma_start(w[:], wq_d[a])
                            wqt.append(w)
                        for dc in range(8):
                            ps = qkps.tile([P, SL], F32, name=f"qps{dc}",
                                           tag="qk")
                            for a in range(4):
                                nc.tensor.matmul(
                                    ps[:], wqt[a][:, :, dc * P:(dc + 1) * P],
                                    hT8[:, 2 * a:2 * a + 2, :],
                                    start=(a == 0), stop=(a == 3),
                                    perf_mode=DR)
                            nc.vector.tensor_scalar(QT8[:, dc, :], ps[:],
                                                    bq_t[:, dc:dc + 1],
                                                    1.0 / (2.0 * WS),
                                                    OP.add, OP.mult)

                # W1 prefetch (consumed in the FFN, DMAs overlap attention)
                w1t = [[None] * 4 for _ in range(4)]
                for qq in range(4):
                    for a in range(4):
                        w = w1pool.tile([P, 2, F // 4], FP8,
                                        name=f"w1_{qq}_{a}", tag="w1")
                        nc.sync.dma_start(
                            w[:], w1_d[a][:, :, qq * 1024:(qq + 1) * 1024])
                        w1t[qq][a] = w

                # ---------------- phase 4: attention ----------------
                # PSUM holds score = raw/4 (K fp8 x (Q/2) = 8*score/2).
                ctxT8 = ctx_pool.tile([P, 8, SL], BF16)
                with (
                    tc.tile_pool(name="kst", bufs=6) as kst,
                    tc.tile_pool(name="vst", bufs=3) as vst,
                    tc.tile_pool(name="esb", bufs=24) as esb,
                    tc.tile_pool(name="bcs_pool", bufs=2) as bcs_pool,
                    tc.tile_pool(name="rs_pool", bufs=1) as rs_pool,
                    tc.tile_pool(name="spsum", bufs=2, space="PSUM") as spsum,
                    tc.tile_pool(name="cpsum", bufs=2, space="PSUM") as cpsum,
                ):
                    # software-pipelined emission: scores(g+1) is emitted
                    # BEFORE ctx(g) so the in-order PE queue never blocks on
                    # the exp that ctx consumes.
                    cps_all = {}

                    def get_cps(hh, i):
                        key = (hh, i)
                        if key not in cps_all:
                            cps_all[key] = cpsum.tile(
                                [65, SL], F32, name=f"ctx{hh}_{i}",
                                tag=f"ctx{i}")
                        return cps_all[key]

                    kts = {}
                    vt4s = {}

                    def load_k(hh, c):
                        if (hh, c) in kts:
                            return
                        kt = kst.tile([P, SL], FP8, name=f"kt{hh}_{c}",
                                      tag="kt")
                        GK = GK1 if hh < 4 else GK2
                        nc.sync.dma_start(
                            kt[:], GK[c, (hh % 4) * P:(hh % 4 + 1) * P, :])
                        kts[(hh, c)] = kt

                    def load_v(hh, c):
                        if (hh, c) in vt4s:
                            return
                        vt4 = vst.tile([P, 4, 2 * EV], BF16,
                                       name=f"vt{hh}_{c}", tag="vt")
                        GVx = GV1 if hh < 4 else GV2
                        hm = hh % 4
                        nc.sync.dma_start(
                            vt4[:],
                            GVx[c][:, hm * 2 * EV:(hm + 1) * 2 * EV].rearrange(
                                "(kbl p) e -> p kbl e", p=P))
                        vt4s[(hh, c)] = vt4

                    steps = [(hh, c, h01, g)
                             for hh in range(8)
                             for c in range(NCORES)
                             for h01 in range(2)
                             for g in range(2)]
                    pending = None

                    def emit_scores_exp(step):
                        hh, c, h01, g = step
                        load_k(hh, c)
                        kt = kts[(hh, c)]
                        rhs_q = QT8[h01 * 64:(h01 + 1) * 64, hh, :]
                        sps = spsum.tile([P, 1024], F32,
                                         name=f"sp{hh}_{c}_{h01}_{g}",
                                         tag="sp")
                        for kk in range(2):
                            kbl = g * 2 + kk
                            nc.tensor.matmul(
                                sps[:, kk * 512:(kk + 1) * 512],
                                kt[h01 * 64:(h01 + 1) * 64,
                                   kbl * P:(kbl + 1) * P],
                                rhs_q, start=True, stop=True)
                        et = esb.tile([P, 2, 512], BF16,
                                      name=f"e{hh}_{c}_{h01}_{g}", tag="et")
                        nc.scalar.activation(
                            et.rearrange("p a b -> p (a b)"), sps[:],
                            AF.Exp, scale=0.25)
                        return (step, et)

                    def emit_ctx(item):
                        (hh, c, h01, g), et = item
                        load_v(hh, c)
                        vt4 = vt4s[(hh, c)]
                        for kk in range(2):
                            kbl = g * 2 + kk
                            nc.tensor.matmul(
                                get_cps(hh, h01)[:],
                                vt4[:, kbl, h01 * EV:h01 * EV + 65],
                                et[:, kk, :],
                                start=(c == 0 and kbl == 0),
                                stop=(c == 7 and kbl == 3))

                    def _normalize(hh, cps):
                        nc.vector.tensor_copy(rc65_f[0:1, :],
                                              cps[0][64:65, :])
                        nc.vector.tensor_copy(rc65_f[64:65, :],
                                              cps[1][64:65, :])
                        rcf = rs_pool.tile([65, SL], F32, name=f"rcf{hh}",
                                           tag="rcf")
                        nc.vector.reciprocal(rcf[:], rc65_f[:])
                        rc65 = rs_pool.tile([65, SL], F32R, name=f"rc{hh}",
                                            tag="rc")
                        nc.vector.tensor_copy(rc65[:], rcf[:])
                        bcw = spsum.tile([P, 1024], F32, name=f"bc{hh}",
                                         tag="sp")
                        bc = bcw[:, 0:SL]
                        nc.tensor.matmul(bc, e65[:], rc65[:], start=True,
                                         stop=True)
                        bcs = bcs_pool.tile([P, SL], F32, name=f"bcs{hh}",
                                            tag="bcs")
                        nc.vector.tensor_copy(bcs[:], bc)
                        nc.vector.tensor_tensor(ctxT8[0:64, hh, :],
                                                cps[0][0:64, :],
                                                bcs[0:64, :], OP.mult)
                        nc.vector.tensor_tensor(ctxT8[64:128, hh, :],
                                                cps[1][0:64, :],
                                                bcs[64:128, :], OP.mult)

                    def emit_normalize(hh):
                        cps = [cps_all[(hh, 0)], cps_all[(hh, 1)]]
                        _normalize(hh, cps)

                    DEPTH = 20
                    PFV = 6
                    queue = []

                    def do_ctx(item):
                        emit_ctx(item)
                        phh, pc, ph01, pg = item[0]
                        if pc == 7 and ph01 == 1 and pg == 1:
                            emit_normalize(phh)

                    for si, step in enumerate(steps):
                        item = emit_scores_exp(step)
                        queue.append(item)
                        ci = si - DEPTH
                        if ci >= 0:
                            pf = ci + PFV
                            if pf < len(steps):
                                load_v(steps[pf][0], steps[pf][1])
                            do_ctx(queue[ci])
                    for ci in range(len(steps) - DEPTH, len(steps)):
                        do_ctx(queue[ci])

            # ---------------- phase 5: out-proj + residual ----------------
            x2 = x2_pool.tile([P, 4, D], F32)
            with (
                tc.tile_pool(name="wopool", bufs=8) as wopool,
                tc.tile_pool(name="ops", bufs=2, space="PSUM") as opps,
                tc.tile_pool(name="tp2", bufs=2, space="PSUM") as tpp2,
            ):
                wot = []
                for cc in range(8):
                    w = wopool.tile([P, D], BF16, name=f"wo{cc}", tag="wo")
                    nc.sync.dma_start(w[:], wo_d[cc])
                    wot.append(w)
                for sb in range(4):
                    for eb in range(2):
                        ps = opps.tile([P, 512], F32, name=f"op{sb}_{eb}",
                                       tag="op")
                        for cc in range(8):
                            nc.tensor.matmul(
                                ps[:],
                                ctxT8[:, cc, sb * P:(sb + 1) * P],
                                wot[cc][:, eb * 512:(eb + 1) * 512],
                                start=(cc == 0), stop=False)
                        nc.tensor.matmul(ps[:], ones65[32:33, :],
                                         bor[:, eb * 512:(eb + 1) * 512],
                                         start=False, stop=True)
                        nc.vector.tensor_tensor(
                            x2[:, sb, eb * 512:(eb + 1) * 512], ps[:],
                            x_big[:, sb, eb * 512:(eb + 1) * 512], OP.add)
                    ln_j(x2, sb, ln2_a, ln2_b, h2T8, tpp2)

        # ---------------- phase 6: LN2 + transpose ----------------
        if True:
            # ------------- phases 7/8: FFN in two halves -------------
            # PSUM of GEMM2 = 1024 * (relu @ W2); post-scale 1/1024.
            with (
                tc.tile_pool(name="atpool", bufs=2) as atpool,
                tc.tile_pool(name="w2pool", bufs=6) as w2pool,
                tc.tile_pool(name="o2ppool", bufs=1) as o2ppool,
                tc.tile_pool(name="outpool", bufs=3) as outpool,
            ):
                o2p = o2ppool.tile([P, 4, D], F32)
                for half in range(2):
                    with tc.tile_pool(name=f"f1ps{half}", bufs=2,
                                      space="PSUM") as f1ps:
                        at_h = []
                        for qq in range(half * 2, half * 2 + 2):
                            ATq = atpool.tile([P, 8, SL], BF16,
                                              name=f"at{qq}", tag="at")
                            for fc in range(8):
                                fg = qq * 8 + fc
                                ps = f1ps.tile([P, SL], F32, name=f"f1_{fg}",
                                               tag="f1")
                                for a in range(4):
                                    nc.tensor.matmul(
                                        ps[:],
                                        w1t[qq][a][:, :, fc * P:(fc + 1) * P],
                                        h2T8[:, 2 * a:2 * a + 2, :],
                                        start=(a == 0), stop=(a == 3),
                                        perf_mode=DR)
                                # ATq holds 32*relu(h@W1 + b1)
                                nc.vector.tensor_scalar(ATq[:, fc, :], ps[:],
                                                        b1_t[:, fg:fg + 1],
                                                        0.0, OP.add, OP.max)
                            at_h.append(ATq)
                    with tc.tile_pool(name=f"f2ps{half}", bufs=8,
                                      space="PSUM") as f2ps:
                        pss = [f2ps.tile([P, 512], F32,
                                         name=f"f2_{half}_{i}", tag="f2")
                               for i in range(8)]
                        for fcl in range(16):
                            qi, fc = divmod(fcl, 8)
                            gg = half * 16 + fcl
                            w2t = w2pool.tile([P, D], BF16,
                                              name=f"w2_{gg}", tag="w2")
                            nc.sync.dma_start(w2t[:], w2_d[gg])
                            for sb in range(4):
                                for eb in range(2):
                                    nc.tensor.matmul(
                                        pss[sb * 2 + eb][:],
                                        at_h[qi][:, fc, sb * P:(sb + 1) * P],
                                        w2t[:, eb * 512:(eb + 1) * 512],
                                        start=(fcl == 0),
                                        stop=(half == 0 and fcl == 15))
                        for sb in range(4):
                            for eb in range(2):
                                ps = pss[sb * 2 + eb]
                                sl = slice(eb * 512, (eb + 1) * 512)
                                if half == 0:
                                    nc.vector.scalar_tensor_tensor(
                                        o2p[:, sb, sl], ps[:],
                                        1.0 / WS, x2[:, sb, sl],
                                        OP.mult, OP.add)
                                else:
                                    nc.tensor.matmul(ps[:], ones65[64:65, :],
                                                     b2r[:, sl],
                                                     start=False, stop=True)
                                    ot = outpool.tile([P, 512], F32,
                                                      name=f"ot{sb}_{eb}",
                                                      tag="ot")
                                    nc.vector.scalar_tensor_tensor(
                                        ot[:], ps[:], 1.0 / WS,
                                        o2p[:, sb, sl], OP.mult, OP.add)
                                    nc.sync.dma_start(
                                        y_d[sb * P:(sb + 1) * P, sl], ot[:])

    nc.compile()
    return nc


def _prep_w_pairs(w, scale):
    """[D_in, N] f32 -> [D_in//256, 128, 2, N] fp8 contraction pairs."""
    d_in, n = w.shape
    return np.ascontiguousarray(
        (w * scale).reshape(d_in // 256, 2, P, n).transpose(0, 2, 1, 3)
    ).astype(NP_FP8)


def make_in_maps(inp):
    """Build the per-core input maps from full f32 inputs."""
    xf = inp["x"].reshape(S, D)
    shared = {
        "Wq8": _prep_w_pairs(inp["Wq"], WS),
        "Wk8": _prep_w_pairs(inp["Wk"], WS),
        "Wv8": _prep_w_pairs(inp["Wv"], WS),
        "Wo16": np.ascontiguousarray(
            inp["Wo"].reshape(8, P, D)).astype(ml_dtypes.bfloat16),
        "W18": _prep_w_pairs(inp["W1"], WS),
        "W216": np.ascontiguousarray(
            inp["W2"].reshape(32, P, D)).astype(ml_dtypes.bfloat16),
        "bq32": inp["bq"] * WS,
        "bk32": inp["bk"] * WS,
        "bv32": inp["bv"] * WS,
        "bo32": inp["bo"],
        "b1_32": inp["b1"] * WS,
        "b2_1024": inp["b2"] * WS,
    }
    in_maps = []
    for c in range(NCORES):
        m = dict(shared)
        m["x_loc"] = np.ascontiguousarray(xf[c * SL:(c + 1) * SL, :])
        in_maps.append(m)
    return in_maps


def kernel(**inputs):
    inp = {k: np.asarray(v, dtype=np.float32) for k, v in inputs.items()}
    x = inp["x"]
    B = x.shape[0]
    key = (float(inp["ln1_a"][0]), float(inp["ln1_b"][0]),
           float(inp["ln2_a"][0]), float(inp["ln2_b"][0]))
    if key not in _CACHE:
        _CACHE[key] = _build(*key)
    nc = _CACHE[key]

    in_maps = make_in_maps(inp)
    res = run_bass_kernel_spmd(nc, in_maps, list(range(NCORES)))
    out = np.concatenate([res.results[c]["y_loc"] for c in range(NCORES)],
                         axis=0)
    return out.reshape(B, S, D)
